# revision 56
# baseline (speedup 1.0000x reference)
"""Dilated attention kernel for Trainium2 (8 NeuronCores, SPMD).

Problem: B=4, H=8, L=2048, D=128, dilation ratios [1,2,4,8].
Inputs  query/key/value: [32, 2048, 128] f32 (grouped (b h)).
Output: [4, 2048, 1024] f32 (b, l, h*d).

Math: for ratio dr, head h attends within the strided position subset
{p : p % dr == r(h)}; per-ratio results are scatter-added into the output.

Key ideas:

1. sigma trick (from the earlier version): permute positions by
   sigma(p) = rev3(p%8)*256 + p//8. Every (dr, r) gather set becomes a
   CONTIGUOUS row block, so on-device everything is dense attention over
   static row ranges.

2. Score sharing (new): for a fixed head, the ratio-dr score matrix is a
   principal submatrix of the FULL LxL score matrix (the dr=1 one). So we
   compute S = K Q^T once per head, exp once, and run only the extra
   P-block @ V-block matmuls per ratio. This cuts both ScalarE exp work
   (the previous bottleneck) and PE score work by 25%.

3. Sharding: core c = (batch b=c//2, query-half qh=c%2) owns sigma query
   rows [qh*1024, qh*1024+1024) of all 8 heads, with full keys. Each
   ratio-dr block is then either fully inside the half or disjoint from
   it; per core, the per-head "extra ratio" patterns form the same
   multiset {all 8 subsets of {2,4,8}}, so after a per-core head renaming
   plus per-head key-chunk / query-chunk permutations (all host-side),
   one canonical SPMD program serves all 8 cores.

4. Engine balance: S^T chunks via f32r matmuls (1 cyc/row), exp on
   ScalarE from PSUM in 3-bank groups (N=1536), PV in bf16 with a ones
   column for softmax denominators, DVE normalize (+ ratio accumulate via
   scalar_tensor_tensor), all software-pipelined so PE and ACT both stay
   ~100% busy: PV matmul chains are emitted between S-groups with a
   fill-credit scheduler matched to ACT op durations.
"""

import numpy as np

B, H, L, D = 4, 8, 2048, 128
REV3 = [0, 4, 2, 6, 1, 5, 3, 7]

# sigma and its inverse as row-index arrays
P_OF_PI = np.array([(pi % 256) * 8 + REV3[pi // 256] for pi in range(L)])
SIG = np.empty(L, np.int64)
SIG[P_OF_PI] = np.arange(L)

# program heads in emission order: pattern = which extra ratios this head
# runs (descending PV weight so the tail head is cheap)
PATTERNS = [(2, 4, 8), (2, 4), (2, 8), (2,), (4, 8), (4,), (8,), ()]

# canonical layout per pattern: dr -> (m-chunk0, nM, l-chunk0, nL)
CANON = {
    (2, 4, 8): {2: (0, 8, 0, 8), 4: (0, 4, 0, 4), 8: (0, 2, 0, 2)},
    (2, 4): {2: (0, 8, 0, 8), 4: (0, 4, 0, 4)},
    (2, 8): {2: (0, 8, 0, 8), 8: (0, 2, 0, 2)},
    (2,): {2: (0, 8, 0, 8)},
    (4, 8): {4: (0, 4, 0, 4), 8: (4, 2, 4, 2)},
    (4,): {4: (0, 4, 0, 4)},
    (8,): {8: (0, 2, 0, 2)},
    (): {},
}


def _rev(x, nbits):
    r = 0
    for i in range(nbits):
        r |= ((x >> i) & 1) << (nbits - 1 - i)
    return r


def _off(dr, h):
    """sigma-space row offset of the (dr, r(h)) block."""
    ld = dr.bit_length() - 1
    r = h >> (3 - ld)
    return _rev(r, ld) * (L // dr)


def _phys_head(pattern, qh):
    b2 = qh if 2 in pattern else 1 - qh
    b1 = qh if 4 in pattern else 1 - qh
    b0 = qh if 8 in pattern else 1 - qh
    return (b2 << 2) | (b1 << 1) | b0


def _perms(pattern, qh):
    """(phys head, key-chunk perm[16], query-chunk perm[8]) mapping the
    canonical program layout to physical sigma-space chunks."""
    h = _phys_head(pattern, qh)
    lay = CANON[pattern]
    pm = [None] * 16
    used = set()
    for dr in (8, 4, 2):
        if dr not in lay:
            continue
        m0, nM = lay[dr][0], lay[dr][1]
        c0 = _off(dr, h) // 128
        phys = list(range(c0, c0 + (L // dr) // 128))
        targ = [t for t in range(m0, m0 + nM) if pm[t] is None]
        avail = [c for c in phys if c not in used]
        assert len(targ) == len(avail)
        for t, c in zip(targ, avail):
            pm[t] = c
            used.add(c)
    rest = [c for c in range(16) if c not in used]
    for t in range(16):
        if pm[t] is None:
            pm[t] = rest.pop(0)

    pq = [None] * 8
    usedq = set()
    for dr in (8, 4):
        if dr not in lay:
            continue
        l0, nL = lay[dr][2], lay[dr][3]
        q0 = (_off(dr, h) - qh * 1024) // 128
        assert 0 <= q0 and q0 + nL <= 8
        phys = list(range(q0, q0 + nL))
        targ = [t for t in range(l0, l0 + nL) if pq[t] is None]
        avail = [c for c in phys if c not in usedq]
        assert len(targ) == len(avail)
        for t, c in zip(targ, avail):
            pq[t] = c
            usedq.add(c)
    restq = [c for c in range(8) if c not in usedq]
    for t in range(8):
        if pq[t] is None:
            pq[t] = restq.pop(0)
    return h, pm, pq


PERMS = {(qh, p): _perms(p, qh) for qh in (0, 1) for p in PATTERNS}

_CACHE = {}

# build-time tuning knobs
CFG = {
    "groups": (3, 3, 3, 3, 2, 2),  # chunk grouping per 512-l strip (ACT op sizes)
    "groups0": (1, 2, 3, 3, 3, 2, 2),  # head-0 strip-0 grouping (fast start)
    "ps_w": 3,                     # psS tile width (banks); bufs=2 -> 6 banks
    "ps_s_bufs": 2,
    "ps_o_bufs": 2,
    "pt_bufs": 3,
    "work_bufs": 2,
    "load_ahead": 1,
    "fill_eps": 10.0,             # ns reserved per S-group for sem latency
    "fill_eps_early": 120.0,       # conservative eps while PV supply is scarce
    "early_heads": 0,
    "cap_early": 1000.0,           # tighter debt cap while PV supply is scarce
    "cap_early_heads": 0,              # heads using fill_eps_early
    "fill_cap": 2800.0,            # max accumulated fill debt (ns)
    "fill_over": 150.0,            # allowed overshoot when best-fitting
    "fill_min": 225.0,             # emit smallest chain if debt above this
    "taper_head": 8,               # drop fill_eps from this head on (drain tail)
    "tail_split": False,            # split last head s1 into 2x256-l strips
    "warmup_mms": 8,              # dummy matmuls to lift PE out of cold pstate
    "store_chunks": 2,             # output store granularity (l-chunks)
}

# cost-model constants for the fill scheduler (ns)
_ACT_NS = lambda n: (n + 222) / 1.2
_PE_MM = lambda n: n / 2.4


def _build():
    """Build + compile the SPMD Bass program (identical on all 8 cores)."""
    import concourse.bass as bass
    import concourse.mybir as mybir
    import concourse.tile as tile
    from concourse import bacc

    f32 = mybir.dt.float32
    f32r = mybir.dt.float32r
    bf16 = mybir.dt.bfloat16

    nc = bacc.Bacc()
    qt = nc.dram_tensor("qt", [H, D, 1024], f32r, kind="ExternalInput")
    kt = nc.dram_tensor("kt", [H, D, L], f32r, kind="ExternalInput")
    vb = nc.dram_tensor("vb", [H, L, 132], bf16, kind="ExternalInput")
    o = nc.dram_tensor("o", [H, 1024, D], f32, kind="ExternalOutput")

    GROUPS = CFG["groups"]
    PSW = CFG["ps_w"]
    assert sum(GROUPS) == 16 and max(GROUPS) <= PSW
    assert sum(CFG["groups0"]) == 16 and max(CFG["groups0"]) <= PSW

    with tile.TileContext(nc) as tc:
        with (
            tc.tile_pool(name="singles", bufs=1) as singles,
            tc.tile_pool(name="work", bufs=CFG["work_bufs"]) as work,
            tc.tile_pool(name="pt_pool", bufs=CFG["pt_bufs"]) as pt_pool,
            tc.tile_pool(name="small", bufs=8) as small,
            tc.tile_pool(name="ps_s", bufs=CFG["ps_s_bufs"], space="PSUM") as ps_s,
            tc.tile_pool(name="ps_o", bufs=CFG["ps_o_bufs"], space="PSUM") as ps_o,
        ):
            # constant bias for exp(s - 20): keeps exp values in fp32/bf16
            # range without a data-dependent row max (|s| <= ~70)
            exp_bias = singles.tile([128, 1], f32)
            nc.vector.memset(exp_bias, -20.0)

            # warm-up: dummy matmuls on a zeroed tile keep the PE pstate
            # ramp running while the first loads are in flight, so the real
            # S-matmuls start at (or close to) full clock
            if CFG["warmup_mms"]:
                wz = singles.tile([128, 512], bf16)
                nc.vector.memset(wz, 0.0)
                wps = ps_s.tile([128, PSW, 512], f32, tag="psS")
                for i in range(CFG["warmup_mms"]):
                    nc.tensor.matmul(
                        wps[:, i % PSW, :],
                        lhsT=wz[:, 0:128],
                        rhs=wz,
                        start=True,
                        stop=True,
                    ).annotate("warmup")

            loads = []
            strips = []  # (j, s, sgroups, wave)
            SLICE_LEFT = {}
            SLICE_STORE = {}

            for j, pattern in enumerate(PATTERNS):
                QT = work.tile([128, 1024], f32r, tag="QT")
                KT = work.tile([128, 16, 128], f32r, tag="KT")
                vbf = work.tile([128, 16, 132], bf16, tag="vbf")
                ostage = work.tile([128, 8, 128], f32, tag="ostage")
                PT = pt_pool.tile([128, 16, 1024], bf16, tag="PT")

                def load(j=j, QT=QT, KT=KT, vbf=vbf):
                    def dk(a, b):
                        nc.sync.dma_start(
                            out=KT[:, a:b, :].rearrange("d c l -> d (c l)"),
                            in_=kt[j, :, a * 128 : b * 128],
                        )

                    def dq(a, b):
                        nc.sync.dma_start(out=QT[:, a:b], in_=qt[j, :, a:b])

                    def dv(a, b):
                        # v ships with its ones column (col 128) baked in
                        nc.sync.dma_start(
                            out=vbf[:, a:b, :],
                            in_=vb[j, a * 128 : b * 128].rearrange(
                                "(c p) d -> p c d", p=128
                            ),
                        )

                    if j == 0:
                        # head 0's tiles have no reuse-WAR, so v pieces can
                        # interleave early; the first S-group (1 chunk) only
                        # needs dq(0,512) + dk(0,1)
                        dq(0, 512)
                        dk(0, 1)
                        dk(1, 4)
                        dv(0, 6)
                        dk(4, 8)
                        dv(6, 11)
                        dk(8, 12)
                        dv(11, 16)
                        dq(512, 1024)
                        dk(12, 16)
                    else:
                        # all k/q pieces strictly before any v piece: the
                        # v-tile write waits on head j-2's PV chains (tile
                        # WAR), and k/q pieces queued behind it on the same
                        # in-order DMA queue would deadlock against S-groups
                        # already emitted on PE. Same reason the ones-column
                        # memset and the output stores avoid the DVE/SP
                        # queues.
                        dk(0, 3)
                        dq(0, 512)
                        dk(3, 8)
                        dk(8, 12)
                        dq(512, 1024)
                        dk(12, 16)
                        dv(0, 6)
                        dv(6, 11)
                        dv(11, 16)

                loads.append(load)

                def make_sgroup(l0, lw, mc0, mc1, j=j, QT=QT, KT=KT, PT=PT):
                    n = mc1 - mc0

                    def emit():
                        ps = ps_s.tile([128, PSW, 512], f32, tag="psS")
                        for i in range(n):
                            nc.tensor.matmul(
                                ps[:, i, 0:lw],
                                lhsT=KT[:, mc0 + i, :],
                                rhs=QT[:, l0 : l0 + lw],
                                start=True,
                                stop=True,
                            ).annotate(f"S j{j} l{l0} mc{mc0 + i}")
                        nc.scalar.activation(
                            out=PT[:, mc0:mc1, l0 : l0 + lw],
                            in_=ps[:, 0:n, 0:lw],
                            func=mybir.ActivationFunctionType.Exp,
                            bias=exp_bias,
                            scale=1.0,
                        ).annotate(f"ACT j{j} l{l0} mc{mc0}:{mc1}")

                    return emit, _PE_MM(n * lw), _ACT_NS(n * lw)

                def make_chain(m0, nM, lc, first, j=j, PT=PT, vbf=vbf,
                               ostage=ostage):
                    def emit():
                        psO = ps_o.tile([128, 132], f32, tag="psO")
                        for i in range(nM):
                            nc.tensor.matmul(
                                psO[:, 0:129],
                                lhsT=PT[:, m0 + i, lc * 128 : (lc + 1) * 128],
                                rhs=vbf[:, m0 + i, 0:129],
                                start=(i == 0),
                                stop=(i == nM - 1),
                            ).annotate(f"PV j{j} lc{lc} m{m0 + i}/{nM}")
                        rec = small.tile([128, 1], f32, tag="rec")
                        nc.vector.reciprocal(rec, psO[:, 128:129])
                        if first:
                            nc.vector.tensor_scalar_mul(
                                ostage[:, lc, :], psO[:, 0:128], rec
                            )
                        else:
                            nc.vector.scalar_tensor_tensor(
                                ostage[:, lc, :],
                                psO[:, 0:128],
                                rec,
                                ostage[:, lc, :],
                                mybir.AluOpType.mult,
                                mybir.AluOpType.add,
                            )

                    return emit

                def make_store(c0, c1, j=j, ostage=ostage):
                    def emit():
                        nc.sync.dma_start(
                            out=o[j, c0 * 128 : c1 * 128, :].rearrange(
                                "(c p) d -> p c d", p=128
                            ),
                            in_=ostage[:, c0:c1, :],
                        )

                    return emit

                # chain records: (j, lc, first, pe_ns, emit)
                lay = CANON[pattern]
                descs = [(0, 16, lc, True) for lc in range(8)]
                for dr in (2, 4, 8):
                    if dr in lay:
                        m0, nM, l0, nL = lay[dr]
                        for lc in range(l0, l0 + nL):
                            descs.append((m0, nM, lc, False))
                recs = [
                    dict(j=j, lc=lc, first=first, pe=_PE_MM(nM * 129),
                         emit=make_chain(m0, nM, lc, first))
                    for (m0, nM, lc, first) in descs
                ]
                # per-slice outstanding-chain counts + store closures
                SC = CFG["store_chunks"]
                slice_left = {}
                slice_store = {}
                for p in range(8 // SC):
                    cs = [r for r in recs if p * SC <= r["lc"] < (p + 1) * SC]
                    slice_left[(j, p)] = len(cs)
                    slice_store[(j, p)] = make_store(p * SC, (p + 1) * SC)
                for r in recs:
                    r["slice"] = (j, r["lc"] // SC)
                SLICE_LEFT.update(slice_left)
                SLICE_STORE.update(slice_store)

                # wave 0: every chain whose l-chunk lies in strip 0 (its
                # PT slice is fully exp'd once strip 0's ACTs ran, even for
                # dr-block chains); wave 1: chains needing strip 1
                w0 = [r for r in recs if r["lc"] < 4]
                w1 = [r for r in recs if r["lc"] >= 4]
                specs = [(0, 512, w0), (512, 512, w1)]
                if pattern == () and CFG["tail_split"]:
                    # last head: narrow strips unlock its dr1 chains sooner,
                    # shrinking the forced post-ACT tail (ACT has end slack)
                    w1a = [r for r in w1 if r["lc"] < 6]
                    w1b = [r for r in w1 if r["lc"] >= 6]
                    specs = [(0, 512, w0), (512, 256, w1a), (768, 256, w1b)]
                for si, (l0, lw, wave) in enumerate(specs):
                    groups = GROUPS
                    if j == 0 and si == 0:
                        # small first group: S starts after only 1 KT chunk
                        groups = CFG["groups0"]
                    sgroups = []
                    mc0 = 0
                    for g in groups:
                        sgroups.append(make_sgroup(l0, lw, mc0, mc0 + g))
                        mc0 += g
                    strips.append((j, si, sgroups, wave))

            # ---- software-pipelined emission ----
            # S-groups stream at ACT-op rate (psS double buffer). The PE-time
            # budget for PV fill between S-group k and k+1 is
            # act(k) - pe(k+1) - eps; chains are picked best-fit from the
            # ready queue so ACT never starves and PE never idles at the
            # psS WAR. A chain for (j, lc, dr-block) is eligible once the
            # dr1 chain for the same (j, lc) has been emitted (ostage
            # first-write ordering); stores are emitted as soon as the last
            # chain of an output slice has run.
            LA = CFG["load_ahead"]
            for j0 in range(min(1 + LA, H)):
                loads[j0]()
            queue = []
            dr1_done = set()
            debt = 0.0

            def eligible(r):
                return r["first"] or (r["j"], r["lc"]) in dr1_done

            def emit_chain(r):
                r["emit"]()
                if r["first"]:
                    dr1_done.add((r["j"], r["lc"]))
                sl = r["slice"]
                SLICE_LEFT[sl] -= 1
                if SLICE_LEFT[sl] == 0:
                    SLICE_STORE[sl]()

            def pop_fill():
                nonlocal debt
                while queue and debt > 0:
                    cands = [r for r in queue if eligible(r)]
                    if not cands:
                        break
                    fit = [r for r in cands
                           if r["pe"] <= debt + CFG["fill_over"]]
                    if fit:
                        r = max(fit, key=lambda r: r["pe"])
                    elif debt >= CFG["fill_min"]:
                        r = min(cands, key=lambda r: r["pe"])
                    else:
                        break
                    queue.remove(r)
                    emit_chain(r)
                    debt -= r["pe"]

            flat = []
            for j, s, sgroups, wave in strips:
                for gi, g in enumerate(sgroups):
                    flat.append((g, j, s, gi == 0, wave if gi == len(sgroups) - 1 else None))
            for idx, ((emit, pe_ns, act_ns), j, s, is_first, wave) in enumerate(flat):
                if is_first and s == 0 and 1 <= j and j + LA < H:
                    loads[j + LA]()
                emit()
                next_pe = flat[idx + 1][0][1] if idx + 1 < len(flat) else 0.0
                eps = CFG["fill_eps"] if j < CFG["taper_head"] else 0.0
                if j < CFG["early_heads"]:
                    eps = CFG["fill_eps_early"]
                cap = (CFG["cap_early"] if j < CFG["cap_early_heads"]
                       else CFG["fill_cap"])
                debt = min(debt + act_ns - next_pe - eps, cap)
                pop_fill()
                if wave is not None:
                    queue.extend(wave)
            debt = float("inf")
            while queue:
                before = len(queue)
                pop_fill()
                if len(queue) == before:
                    raise AssertionError("stuck chain queue")

    nc.compile()
    return nc


def _get_nc():
    if "nc" not in _CACHE:
        _CACHE["nc"] = _build()
    return _CACHE["nc"]


def _make_in_maps(query, key, value):
    import ml_dtypes

    q = query.reshape(B, H, L, D)[:, :, P_OF_PI, :]
    k = key.reshape(B, H, L, D)[:, :, P_OF_PI, :]
    v = value.reshape(B, H, L, D)[:, :, P_OF_PI, :]
    kT = np.ascontiguousarray(k.transpose(0, 1, 3, 2))  # [B,H,D,L]
    vbh = np.zeros((B, H, L, 132), ml_dtypes.bfloat16)   # [B,H,L,D+ones+pad]
    vbh[:, :, :, 0:128] = v
    vbh[:, :, :, 128] = 1.0
    in_maps = []
    base = np.arange(128)
    for c in range(8):
        b, qh = c // 2, c % 2
        qtc = np.empty((H, D, 1024), np.float32)
        ktc = np.empty((H, D, L), np.float32)
        vbc = np.empty((H, L, 132), ml_dtypes.bfloat16)
        for jj, pattern in enumerate(PATTERNS):
            h, pm, pq = PERMS[(qh, pattern)]
            mrows = (np.asarray(pm)[:, None] * 128 + base).ravel()
            qrows = qh * 1024 + (np.asarray(pq)[:, None] * 128 + base).ravel()
            ktc[jj] = kT[b, h][:, mrows]
            vbc[jj] = vbh[b, h][mrows, :]
            qtc[jj] = q[b, h][qrows, :].T
        in_maps.append({"qt": qtc, "kt": ktc, "vb": vbc})
    return in_maps


def _assemble(results):
    total_sig = np.empty((B, H, L, D), np.float32)
    base = np.arange(128)
    for c in range(8):
        b, qh = c // 2, c % 2
        oc = results[c]["o"]  # [H, 1024, 128]
        for jj, pattern in enumerate(PATTERNS):
            h, pm, pq = PERMS[(qh, pattern)]
            qrows = qh * 1024 + (np.asarray(pq)[:, None] * 128 + base).ravel()
            total_sig[b, h, qrows, :] = oc[jj]
    total = total_sig[:, :, SIG, :]
    return np.ascontiguousarray(
        total.transpose(0, 2, 1, 3).reshape(B, L, H * D)
    )


def _run(query, key, value, trace=False, **trace_kwargs):
    from concourse.bass_utils import run_bass_kernel_spmd

    nc = _get_nc()
    in_maps = _make_in_maps(query, key, value)
    res = run_bass_kernel_spmd(
        nc, in_maps, list(range(8)), trace=trace, **trace_kwargs
    )
    return _assemble(res.results), res


def kernel(query, key, value):
    # accept any array-like (np, jax, lists) and normalize to f32 numpy
    query = np.asarray(query, dtype=np.float32)
    key = np.asarray(key, dtype=np.float32)
    value = np.asarray(value, dtype=np.float32)

    # The axon-tunneled devices occasionally drop or desync a dispatch -
    # sometimes with a transient NRT error (retry on exception), sometimes
    # SILENTLY returning corrupted buffers for a core. Execution is
    # deterministic, so dispatch repeatedly until two results agree and
    # return the agreed output.
    import time

    last_err = None
    outs = []
    for attempt in range(6):
        try:
            out, _ = _run(query, key, value)
        except Exception as e:  # noqa: BLE001 - deliberate broad retry
            last_err = e
            time.sleep(3 * (attempt + 1))
            continue
        for prev in outs:
            if np.allclose(prev, out, rtol=0, atol=1e-5, equal_nan=True):
                return out
        outs.append(out)
    if outs:
        return outs[-1]
    raise last_err
